# revision 1
# baseline (speedup 1.0000x reference)
"""Trainium2 Bass kernel for nn_Decoder (ragged_sequence).

Computes: sigmas = x@W_sig + b_sig; h = swish(x@W1 + b1); y = h@W2 + b2;
then per-segment gaussian smoothing (5 equal segments of 20000, window
10000, sigma ~ 200) of y, concatenated.

Strategy (8 NeuronCores, SPMD, full I/O):
  - Host computes the tiny parts (sigmas, h, gaussian windows) in numpy.
  - Output vector N=100000 is sharded over 8 cores (12500 each, rounded
    out to 99 blocks of 128). Each core GEMMs its slice of W2 columns
    (plus +-6 blocks of halo, zero-padded where the halo crosses a
    segment/global boundary) against h, producing y in a [128, cols]
    block layout (partition = position % 128).
  - The gaussian conv is applied as 13 shifted Toeplitz 128x128 matmuls
    accumulated in PSUM (window truncated to +-895 taps; sigma~200 so
    truncated relative error ~4e-4). Segment boundaries inside a core's
    range are handled with left/right input masks + left/right tap
    tiles + an output select -- all host-built data, so one uniform
    SPMD program serves all cores.
"""

import os
from contextlib import ExitStack

import numpy as np

import ml_dtypes

import concourse.bass as bass
import concourse.mybir as mybir
import concourse.tile as tile
from concourse import bacc
from concourse.bass_utils import run_bass_kernel_spmd

# ---------------------------------------------------------------- constants
D = 128
H = 512
N = 100000
NSIG = 5
WIN = 10000          # reference window size
SEGL = 20000         # segment length
NCORES = 8
PER = N // NCORES    # 12500 outputs per core
BLK = 128
OUTB = 99            # output blocks per core (99*128 = 12672 >= 12500 + max misalign 84)
HB = 6               # halo blocks on each side (+-768 positions)
EXTB = OUTB + 2 * HB # 111 ext blocks of y per core
TAPB = 2 * HB + 1    # 13 Toeplitz shift tiles
CHUNKS = [4, 4, 8, 8, 12, 16, 16, 16, 16, 11]  # W2 column-block chunks (sum = EXTB)
KCH = H // BLK       # 4 contraction chunks

assert sum(CHUNKS) == EXTB

BSTART = [(k * PER) // BLK for k in range(NCORES)]

_DT = os.environ.get("BASS_DECODER_DTYPE", "bf16")

_CACHED_NC = {}


def _np_dt(dt):
    return ml_dtypes.bfloat16 if dt == "bf16" else np.float32


def _mybir_dt(dt):
    return mybir.dt.bfloat16 if dt == "bf16" else mybir.dt.float32


# ---------------------------------------------------------------- device IR
def _build_nc(dt: str):
    """Build + compile the SPMD Bass kernel (same program for all cores)."""
    if dt in _CACHED_NC:
        return _CACHED_NC[dt]
    f32 = mybir.dt.float32
    dtc = _mybir_dt(dt)

    nc = bacc.Bacc(
        "TRN2",
        target_bir_lowering=False,
        debug=False,
        enable_asserts=False,
        num_devices=NCORES,
    )
    ins = {}

    def din(name, shape, dtt):
        ins[name] = nc.dram_tensor(name, shape, dtt, kind="ExternalInput").ap()

    din("w2e", [H, EXTB * BLK], dtc)
    din("h", [BLK, KCH], dtc)
    din("b2e", [BLK, EXTB], f32)
    din("tl", [BLK, TAPB * BLK], dtc)
    din("tr", [BLK, TAPB * BLK], dtc)
    din("ml", [BLK, EXTB], f32)
    din("mr", [BLK, EXTB], f32)
    din("oml", [BLK, OUTB], f32)
    din("omr", [BLK, OUTB], f32)
    out_ap = nc.dram_tensor("out", [BLK, OUTB], f32, kind="ExternalOutput").ap()

    with tile.TileContext(nc) as tc:
        with ExitStack() as ctx:
            const = ctx.enter_context(tc.tile_pool(name="const", bufs=1))
            wpool = ctx.enter_context(tc.tile_pool(name="w", bufs=6))
            sbp = ctx.enter_context(tc.tile_pool(name="sb", bufs=1))
            pgp = ctx.enter_context(tc.tile_pool(name="pg", bufs=3, space="PSUM"))
            pcp = ctx.enter_context(tc.tile_pool(name="pc", bufs=1, space="PSUM"))

            # constants go on non-Sync DMA queues so the W2 stream starts
            # immediately
            h_sb = const.tile([BLK, KCH], dtc)
            nc.gpsimd.dma_start(h_sb[:], ins["h"][:])
            b2_sb = const.tile([BLK, EXTB], f32)
            nc.scalar.dma_start(b2_sb[:], ins["b2e"][:])
            ml_sb = const.tile([BLK, EXTB], f32)
            nc.scalar.dma_start(ml_sb[:], ins["ml"][:])
            mr_sb = const.tile([BLK, EXTB], f32)
            nc.scalar.dma_start(mr_sb[:], ins["mr"][:])
            tl_sb = const.tile([BLK, TAPB * BLK], dtc)
            nc.gpsimd.dma_start(tl_sb[:], ins["tl"][:])
            tr_sb = const.tile([BLK, TAPB * BLK], dtc)
            nc.gpsimd.dma_start(tr_sb[:], ins["tr"][:])
            oml_sb = const.tile([BLK, OUTB], f32)
            nc.scalar.dma_start(oml_sb[:], ins["oml"][:])
            omr_sb = const.tile([BLK, OUTB], f32)
            nc.scalar.dma_start(omr_sb[:], ins["omr"][:])

            s_sb = sbp.tile([BLK, EXTB], f32)
            sl_sb = sbp.tile([BLK, EXTB], dtc)
            sr_sb = sbp.tile([BLK, EXTB], dtc)

            # ---- GEMM: y[c*128+q] = sum_k h[k] * W2[k, c*128+q]  (+ b2)
            # four plain 2D DMAs per chunk on one queue sustain ~400 GB/s
            # (a single 3D k-interleaved DMA only reaches ~215 GB/s)
            c0 = 0
            for ci, cb_n in enumerate(CHUNKS):
                wts = []
                for kc in range(KCH):
                    wt = wpool.tile([BLK, cb_n * BLK], dtc, tag=f"w{kc}")
                    nc.sync.dma_start(
                        wt[:],
                        ins["w2e"][
                            kc * BLK : (kc + 1) * BLK,
                            c0 * BLK : (c0 + cb_n) * BLK,
                        ],
                    )
                    wts.append(wt)
                psum_g = pgp.tile([BLK, cb_n], f32, tag="pg")
                for cb in range(cb_n):
                    for kc in range(KCH):
                        nc.tensor.matmul(
                            psum_g[:, cb : cb + 1],
                            lhsT=wts[kc][:, cb * BLK : (cb + 1) * BLK],
                            rhs=h_sb[:, kc : kc + 1],
                            start=(kc == 0),
                            stop=(kc == KCH - 1),
                        )
                # finalize this chunk's S columns (bias + boundary masks)
                sl = slice(c0, c0 + cb_n)
                nc.vector.tensor_add(s_sb[:, sl], psum_g[:], b2_sb[:, sl])
                nc.vector.tensor_mul(sl_sb[:, sl], s_sb[:, sl], ml_sb[:, sl])
                nc.vector.tensor_mul(sr_sb[:, sl], s_sb[:, sl], mr_sb[:, sl])
                c0 += cb_n

            # ---- conv: out[q,c] = sum_e sum_p T[e][p,q] * S[p, c+HB+e]
            pA = pcp.tile([BLK, OUTB], f32, tag="pA")
            pB = pcp.tile([BLK, OUTB], f32, tag="pB")
            for ei in range(TAPB):
                nc.tensor.matmul(
                    pA[:, :],
                    lhsT=tl_sb[:, ei * BLK : (ei + 1) * BLK],
                    rhs=sl_sb[:, ei : ei + OUTB],
                    start=(ei == 0),
                    stop=(ei == TAPB - 1),
                )
            for ei in range(TAPB):
                nc.tensor.matmul(
                    pB[:, :],
                    lhsT=tr_sb[:, ei * BLK : (ei + 1) * BLK],
                    rhs=sr_sb[:, ei : ei + OUTB],
                    start=(ei == 0),
                    stop=(ei == TAPB - 1),
                )

            t1 = sbp.tile([BLK, OUTB], f32)
            nc.vector.tensor_mul(t1[:], pA[:], oml_sb[:])
            t2 = sbp.tile([BLK, OUTB], f32)
            nc.vector.tensor_mul(t2[:], pB[:], omr_sb[:])
            o_sb = sbp.tile([BLK, OUTB], f32)
            nc.vector.tensor_add(o_sb[:], t1[:], t2[:])
            nc.sync.dma_start(out_ap[:], o_sb[:])

    nc.compile()
    _CACHED_NC[dt] = nc
    return nc


# ---------------------------------------------------------------- host prep
def _prep_inputs(x, W_sig, b_sig, W1, b1, W2, b2, dt: str):
    npdt = _np_dt(dt)
    f64 = np.float64

    # tiny head + MLP hidden layer on host
    sig = x.astype(f64) @ W_sig.astype(f64) + b_sig.astype(f64)       # [5]
    pre = x.astype(f64) @ W1.astype(f64) + b1.astype(f64)             # [512]
    h = pre / (1.0 + np.exp(-pre))                                    # swish

    # normalized gaussian taps per segment: G_s(m) = exp(-m^2/2s^2)/Z_s
    # (Z over the full reference window t=0..9999 centered at 5000)
    t = np.arange(WIN, dtype=f64)
    Z = np.exp(-((t[None, :] - WIN / 2) ** 2) / (2 * sig[:, None] ** 2)).sum(axis=1)

    p = np.arange(BLK)[:, None]
    q = np.arange(BLK)[None, :]
    e = np.arange(-HB, HB + 1)[:, None, None]
    m = e * BLK + p[None] - q[None] + 1                               # [13,128,128]
    tiles = []
    for s in range(NSIG):
        g = np.exp(-(m.astype(f64) ** 2) / (2 * sig[s] ** 2)) / Z[s]
        tiles.append(np.ascontiguousarray(g.transpose(1, 0, 2)).reshape(BLK, -1))

    h_in = np.ascontiguousarray(h.reshape(KCH, BLK).T).astype(npdt)

    in_maps = []
    meta = []
    for k in range(NCORES):
        lo = (BSTART[k] - HB) * BLK
        hi = lo + EXTB * BLK
        out0 = BSTART[k] * BLK
        glo, ghi = max(lo, 0), min(hi, N)

        w2e = np.zeros((H, EXTB * BLK), dtype=npdt)
        w2e[:, glo - lo : ghi - lo] = W2[:, glo:ghi].astype(npdt)
        b2p = np.zeros(EXTB * BLK, dtype=np.float32)
        b2p[glo - lo : ghi - lo] = b2[glo:ghi]
        b2e = np.ascontiguousarray(b2p.reshape(EXTB, BLK).T)

        B = None
        for b in range(SEGL, N, SEGL):
            if lo < b < hi:
                B = b
        ext_pos = lo + np.arange(EXTB)[None, :] * BLK + np.arange(BLK)[:, None]
        out_pos = out0 + np.arange(OUTB)[None, :] * BLK + np.arange(BLK)[:, None]
        if B is None:
            seg = min(out0 // SEGL, NSIG - 1)
            tl = tr = tiles[seg]
            ml = np.ones((BLK, EXTB), np.float32)
            mr = np.zeros((BLK, EXTB), np.float32)
            oml = np.ones((BLK, OUTB), np.float32)
            omr = np.zeros((BLK, OUTB), np.float32)
        else:
            tl = tiles[B // SEGL - 1]
            tr = tiles[B // SEGL]
            ml = (ext_pos < B).astype(np.float32)
            mr = (ext_pos >= B).astype(np.float32)
            oml = (out_pos < B).astype(np.float32)
            omr = (out_pos >= B).astype(np.float32)

        in_maps.append(
            {
                "w2e": w2e,
                "h": h_in,
                "b2e": b2e,
                "tl": np.ascontiguousarray(tl).astype(npdt),
                "tr": np.ascontiguousarray(tr).astype(npdt),
                "ml": ml,
                "mr": mr,
                "oml": oml,
                "omr": omr,
            }
        )
        meta.append((out0, k * PER - out0))
    return in_maps, meta


def _assemble(results, meta):
    full = np.empty(N, dtype=np.float32)
    for k in range(NCORES):
        arr = results[k]["out"]                         # [128, OUTB]
        flat = np.ascontiguousarray(arr.T).reshape(-1)  # pos out0 + i
        off = meta[k][1]
        full[k * PER : (k + 1) * PER] = flat[off : off + PER]
    return full


def run_with_results(inputs: dict, dt: str | None = None, trace: bool = False):
    dt = dt or _DT
    args = {k: np.asarray(v, dtype=np.float32) for k, v in inputs.items()}
    in_maps, meta = _prep_inputs(
        args["x"], args["W_sig"], args["b_sig"], args["W1"], args["b1"],
        args["W2"], args["b2"], dt,
    )
    nc = _build_nc(dt)
    res = run_bass_kernel_spmd(
        nc, in_maps, core_ids=list(range(NCORES)), trace=trace
    )
    return _assemble(res.results, meta), res


def kernel(**inputs) -> np.ndarray:
    out, _ = run_with_results(inputs)
    return out

